# revision 5
# baseline (speedup 1.0000x reference)
"""Multi-head self-attention (no mask) on 8 TRN2 NeuronCores.

Sharding: tensor-parallel over heads (2 heads/core) for QKV + attention,
then an AllToAll re-shards to row-parallel for the output projection.

Per-core dataflow:
  A) qT,kT = W{q,k}_c @ x.T (transposed layout), v = x @ Wv_c.T (natural).
     Inputs cast to bf16 on DVE; matmuls in bf16, f32 PSUM accum.
  B) per (head, batch): scoresT = k q^T, expT = exp(scale*scoresT)
     [no max-subtraction: scores are O(5)], attnT_unnorm = v^T expT via PE,
     softmax denominators via a DVE pairwise-add tree over expT k-tiles
     + one ones^T matmul, attnT = attnT_unnorm * broadcast(1/sums).
     One AllToAll per head; payloads in bf16.
  C) out_rows = attn_rows @ Wo.T + bo (bf16 matmuls, f32 accum + bias).

A single tagged PSUM pool (8 banks split across phases) and tight SBUF
budgeting let phases overlap: attention for batch 0 starts while QKV is
still streaming later row-chunks; Wo prefetch runs during attention.
"""

import numpy as np

import concourse.bass as bass
import concourse.tile as tile
from concourse import bacc, mybir
from concourse.bass_utils import run_bass_kernel_spmd

F32 = mybir.dt.float32
F32R = mybir.dt.float32r
BF16 = mybir.dt.bfloat16

B, S, H = 2, 2048, 2048
NH, HD = 16, 128
NC = 8
BS = B * S          # 4096 rows total
FL = H // NC        # 256 features per core (2 heads)
HL = NH // NC       # 2 heads per core
RPC = BS // NC      # 512 output rows per core
K16 = H // 128      # 16 contraction tiles
CW = 512            # phase-A row-chunk width
NCHUNK = BS // CW   # 8
QC = 512            # attention q-chunk width
SCALE = 1.0 / float(np.sqrt(HD))

_CACHED = None


def _build():
    nc = bacc.Bacc("TRN2", target_bir_lowering=False, debug=False, num_devices=NC)

    xT_d = nc.dram_tensor("xT", [H, BS], F32, kind="ExternalInput")
    wqT_d = nc.dram_tensor("wqT", [H, FL], F32, kind="ExternalInput")
    wkT_d = nc.dram_tensor("wkT", [H, FL], F32, kind="ExternalInput")
    wvT_d = nc.dram_tensor("wvT", [H, FL], F32, kind="ExternalInput")
    bq_d = nc.dram_tensor("bq", [128, HL], F32, kind="ExternalInput")
    bk_d = nc.dram_tensor("bk", [128, HL], F32, kind="ExternalInput")
    bv_d = nc.dram_tensor("bv_bc", [128, FL], F32, kind="ExternalInput")
    woT_d = nc.dram_tensor("woT", [H, H], F32, kind="ExternalInput")
    bo_d = nc.dram_tensor("bo_bc", [128, H], F32, kind="ExternalInput")
    ones_d = nc.dram_tensor("ones", [128, 128], F32, kind="ExternalInput")
    onesb_d = nc.dram_tensor("ones_bf", [128, 128], BF16, kind="ExternalInput")
    out_d = nc.dram_tensor("out", [RPC, H], F32, kind="ExternalOutput")

    with tile.TileContext(nc) as tc:
        with (
            tc.tile_pool(name="consts", bufs=1) as cstp,
            tc.tile_pool(name="dram", bufs=1, space="DRAM") as dp,
            tc.tile_pool(name="stg", bufs=1) as stg,
            tc.tile_pool(name="woPre", bufs=1) as wcp,
            tc.tile_pool(name="psum", bufs=1, space="PSUM") as pp,
        ):
            ones_sb = cstp.tile([128, 128], F32R)
            nc.sync.dma_start(ones_sb[:], ones_d.ap()[:].bitcast(F32R))
            ones_bf = cstp.tile([128, 128], BF16)
            nc.sync.dma_start(ones_bf[:], onesb_d.ap()[:])

            a2a_in = [dp.tile([NC, 128, RPC], BF16, name=f"a2a_in{h}") for h in range(HL)]
            a2a_out = [dp.tile([NC, 128, RPC], BF16, name=f"a2a_out{h}") for h in range(HL)]

            def cast_load(dst_slice, src_ap, width):
                """DMA f32 -> small staging, DVE-cast into bf16 dst slice."""
                src = stg.tile([128, CW], F32, tag="stg", bufs=4)
                nc.sync.dma_start(src[:, :width], src_ap)
                nc.vector.tensor_copy(dst_slice, src[:, :width])

            with tc.tile_pool(name="qkv", bufs=1) as qkvp:
                qT_sb = qkvp.tile([128, HL * BS], BF16)
                kT_sb = qkvp.tile([128, HL * BS], BF16)
                v_sb = qkvp.tile([128, (BS // 128) * FL], BF16)

                # ---------------- Phase A: QKV projections ----------------
                with (
                    tc.tile_pool(name="wgt", bufs=1) as wp,
                    tc.tile_pool(name="xbf", bufs=2) as xbp,
                ):
                    def load_w(dram):
                        dst = wp.tile([128, K16 * FL], BF16, tag=f"w_{dram.name}")
                        for k in range(K16):
                            cast_load(
                                dst[:, k * FL:(k + 1) * FL],
                                dram.ap()[k * 128:(k + 1) * 128, :],
                                FL,
                            )
                        return dst

                    def load_x(c):
                        dst = xbp.tile([128, K16 * CW], BF16, tag="xbf")
                        for k in range(K16):
                            cast_load(
                                dst[:, k * CW:(k + 1) * CW],
                                xT_d.ap()[k * 128:(k + 1) * 128,
                                          c * CW:(c + 1) * CW],
                                CW,
                            )
                        return dst

                    wq_sb = load_w(wqT_d)
                    xc0 = load_x(0)
                    wk_sb = load_w(wkT_d)
                    wv_sb = load_w(wvT_d)

                    bq_sb = wp.tile([128, HL], F32)
                    nc.sync.dma_start(bq_sb[:], bq_d.ap()[:])
                    bk_sb = wp.tile([128, HL], F32)
                    nc.sync.dma_start(bk_sb[:], bk_d.ap()[:])
                    bv_sb = wp.tile([128, FL], F32)
                    nc.sync.dma_start(bv_sb[:], bv_d.ap()[:])

                    for c in range(NCHUNK):
                        xc = xc0 if c == 0 else load_x(c)
                        for w_sb, b_sb, dst in (
                            (wq_sb, bq_sb, qT_sb),
                            (wk_sb, bk_sb, kT_sb),
                        ):
                            for m in range(HL):
                                ps = pp.tile([128, CW], F32, tag="psA", bufs=3)
                                for k in range(K16):
                                    nc.tensor.matmul(
                                        ps[:],
                                        w_sb[:, k * FL + m * 128:
                                             k * FL + (m + 1) * 128],
                                        xc[:, k * CW:(k + 1) * CW],
                                        start=(k == 0),
                                        stop=(k == K16 - 1),
                                    )
                                nc.vector.tensor_scalar_add(
                                    dst[:, m * BS + c * CW: m * BS + (c + 1) * CW],
                                    ps[:],
                                    b_sb[:, m:m + 1],
                                )
                        for m2 in range(CW // 128):
                            ps = pp.tile([128, FL], F32, tag="psA", bufs=3)
                            for k in range(K16):
                                nc.tensor.matmul(
                                    ps[:],
                                    xc[:, k * CW + m2 * 128: k * CW + (m2 + 1) * 128],
                                    wv_sb[:, k * FL:(k + 1) * FL],
                                    start=(k == 0),
                                    stop=(k == K16 - 1),
                                )
                            i = c * (CW // 128) + m2
                            nc.vector.tensor_add(
                                v_sb[:, i * FL:(i + 1) * FL], ps[:], bv_sb[:]
                            )

                # prefetch Wo (cast to bf16) during attention
                won_tiles = {}
                for n in range(2):
                    won = wcp.tile([128, K16 * 512], BF16, tag="won", bufs=2)
                    for k in range(K16):
                        cast_load(
                            won[:, k * 512:(k + 1) * 512],
                            woT_d.ap()[k * 128:(k + 1) * 128,
                                       n * 512:(n + 1) * 512],
                            512,
                        )
                    won_tiles[n] = won

                # ---------------- Phase B: attention ----------------
                with (
                    tc.tile_pool(name="expp", bufs=2) as ep,
                    tc.tile_pool(name="tree", bufs=1) as trp,
                    tc.tile_pool(name="attp", bufs=2) as ap_,
                    tc.tile_pool(name="recp", bufs=2) as rp,
                ):
                    for h in range(HL):
                        for b in range(B):
                            base = h * BS + b * S
                            for qc in range(S // QC):
                                dest = b * (S // QC) + qc
                                expT = ep.tile([128, K16 * QC], BF16, tag="expT")
                                for km in range(K16):
                                    pss = pp.tile([128, QC], F32, tag="pss", bufs=2)
                                    nc.tensor.matmul(
                                        pss[:],
                                        kT_sb[:, base + km * 128:
                                              base + (km + 1) * 128],
                                        qT_sb[:, base + qc * QC:
                                              base + (qc + 1) * QC],
                                        start=True,
                                        stop=True,
                                    )
                                    nc.scalar.activation(
                                        expT[:, km * QC:(km + 1) * QC],
                                        pss[:],
                                        mybir.ActivationFunctionType.Exp,
                                        scale=SCALE,
                                    )
                                s1 = trp.tile([128, 8 * QC], BF16, tag="s1")
                                nc.vector.tensor_add(
                                    s1[:], expT[:, :8 * QC], expT[:, 8 * QC:]
                                )
                                s2 = trp.tile([128, 4 * QC], BF16, tag="s2")
                                nc.vector.tensor_add(
                                    s2[:], s1[:, :4 * QC], s1[:, 4 * QC:]
                                )
                                s3 = trp.tile([128, 2 * QC], BF16, tag="s3")
                                nc.vector.tensor_add(
                                    s3[:], s2[:, :2 * QC], s2[:, 2 * QC:]
                                )
                                s4 = trp.tile([128, QC], BF16, tag="s4")
                                nc.vector.tensor_add(s4[:], s3[:, :QC], s3[:, QC:])

                                psa = pp.tile([128, QC], F32, tag="psa", bufs=1)
                                for km in range(K16):
                                    nc.tensor.matmul(
                                        psa[:],
                                        v_sb[:, (16 * b + km) * FL + h * 128:
                                             (16 * b + km) * FL + (h + 1) * 128],
                                        expT[:, km * QC:(km + 1) * QC],
                                        start=(km == 0),
                                        stop=(km == K16 - 1),
                                    )
                                pssum = pp.tile([1, QC], F32, tag="pssum", bufs=1)
                                nc.tensor.matmul(
                                    pssum[:1, :], ones_bf[:, :1], s4[:],
                                    start=True, stop=True,
                                )
                                recip = rp.tile([1, QC], F32R, tag="recip")
                                with nc.allow_low_precision(
                                    reason="fp32r rounding feeds broadcast matmul"
                                ):
                                    nc.vector.reciprocal(recip[:1, :], pssum[:1, :])
                                psb = pp.tile([128, QC], F32, tag="psb", bufs=1)
                                nc.tensor.matmul(
                                    psb[:],
                                    ones_sb[:1, :],
                                    recip[:1, :],
                                    start=True,
                                    stop=True,
                                )
                                rb = rp.tile([128, QC], BF16, tag="rb")
                                nc.vector.tensor_copy(rb[:], psb[:])
                                att = ap_.tile([128, QC], BF16, tag="att")
                                nc.vector.tensor_mul(att[:], psa[:], rb[:])
                                nc.gpsimd.dma_start(
                                    a2a_in[h][dest, :, :], att[:]
                                )

                        nc.gpsimd.collective_compute(
                            "AllToAll",
                            mybir.AluOpType.bypass,
                            ins=[a2a_in[h].opt()],
                            outs=[a2a_out[h].opt()],
                            replica_groups=[list(range(NC))],
                        )

            # ---------------- Phase C: output projection ----------------
            with (
                tc.tile_pool(name="aT", bufs=1) as atp,
                tc.tile_pool(name="boC", bufs=1) as bcp,
                tc.tile_pool(name="outC", bufs=3) as ocp,
            ):
                aT = atp.tile([128, K16 * RPC], BF16)
                for g in range(K16):
                    nc.sync.dma_start(
                        aT[:, g * RPC:(g + 1) * RPC],
                        a2a_out[g % 2][g // 2, :, :],
                    )
                bo_sb = bcp.tile([128, H], F32)
                nc.sync.dma_start(bo_sb[:], bo_d.ap()[:])
                for n in range(H // 512):
                    if n in won_tiles:
                        won = won_tiles[n]
                    else:
                        won = wcp.tile([128, K16 * 512], BF16, tag="won", bufs=2)
                        for k in range(K16):
                            cast_load(
                                won[:, k * 512:(k + 1) * 512],
                                woT_d.ap()[k * 128:(k + 1) * 128,
                                           n * 512:(n + 1) * 512],
                                512,
                            )
                    for m in range(RPC // 128):
                        pso = pp.tile([128, 512], F32, tag="pss", bufs=2)
                        for k in range(K16):
                            nc.tensor.matmul(
                                pso[:],
                                aT[:, k * RPC + m * 128: k * RPC + (m + 1) * 128],
                                won[:, k * 512:(k + 1) * 512],
                                start=(k == 0),
                                stop=(k == K16 - 1),
                            )
                        ot = ocp.tile([128, 512], F32, tag="ot")
                        nc.vector.tensor_add(
                            ot[:], pso[:], bo_sb[:, n * 512:(n + 1) * 512]
                        )
                        nc.sync.dma_start(
                            out_d.ap()[m * 128:(m + 1) * 128,
                                       n * 512:(n + 1) * 512],
                            ot[:],
                        )

    nc.compile()
    return nc


def _get_nc():
    global _CACHED
    if _CACHED is None:
        _CACHED = _build()
    return _CACHED


def _prep_in_maps(x, Wq, bq, Wk, bk, Wv, bv, Wo, bo):
    import ml_dtypes

    xT = np.ascontiguousarray(x.reshape(BS, H).T)
    woT = np.ascontiguousarray(Wo.T)
    bo_bc = np.ascontiguousarray(np.broadcast_to(bo, (128, H)))
    ones = np.ones((128, 128), np.float32)
    ones_bf = np.ones((128, 128), ml_dtypes.bfloat16)
    in_maps = []
    for c in range(NC):
        sl = slice(FL * c, FL * (c + 1))
        in_maps.append(
            {
                "xT": xT,
                "wqT": np.ascontiguousarray(Wq[sl, :].T),
                "wkT": np.ascontiguousarray(Wk[sl, :].T),
                "wvT": np.ascontiguousarray(Wv[sl, :].T),
                "bq": np.ascontiguousarray(bq[sl].reshape(HL, 128).T),
                "bk": np.ascontiguousarray(bk[sl].reshape(HL, 128).T),
                "bv_bc": np.ascontiguousarray(np.broadcast_to(bv[sl], (128, FL))),
                "woT": woT,
                "bo_bc": bo_bc,
                "ones": ones,
                "ones_bf": ones_bf,
            }
        )
    return in_maps


def run(in_maps, trace=False):
    nc = _get_nc()
    return run_bass_kernel_spmd(nc, in_maps, core_ids=list(range(NC)), trace=trace)


def kernel(x, Wq, bq, Wk, bk, Wv, bv, Wo, bo):
    args = [np.asarray(a, dtype=np.float32) for a in (x, Wq, bq, Wk, bk, Wv, bv, Wo, bo)]
    in_maps = _prep_in_maps(*args)
    res = run(in_maps)
    out = np.concatenate([res.results[c]["out"] for c in range(NC)], axis=0)
    return out.reshape(B, S, H)


# revision 6
# speedup vs baseline: 1.1413x; 1.1413x over previous
"""Multi-head self-attention (no mask) on 8 TRN2 NeuronCores.

Sharding: tensor-parallel over heads (2 heads/core) for QKV + attention,
then an AllToAll re-shards to row-parallel for the output projection.

Per-core dataflow:
  A) qT,kT = W{q,k}_c @ x.T (transposed layout), v = x @ Wv_c.T (natural).
     Inputs cast to bf16 on DVE; matmuls in bf16, f32 PSUM accum.
  B) per (head, batch): scoresT = k q^T, expT = exp(scale*scoresT)
     [no max-subtraction: scores are O(5)], attnT_unnorm = v^T expT via PE,
     softmax denominators via a DVE pairwise-add tree over expT k-tiles
     + one ones^T matmul, attnT = attnT_unnorm * broadcast(1/sums).
     One AllToAll per head; payloads in bf16.
  C) out_rows = attn_rows @ Wo.T + bo (bf16 matmuls, f32 accum + bias).

A single tagged PSUM pool (8 banks split across phases) and tight SBUF
budgeting let phases overlap: attention for batch 0 starts while QKV is
still streaming later row-chunks; Wo prefetch runs during attention.
"""

import numpy as np

import concourse.bass as bass
import concourse.tile as tile
from concourse import bacc, mybir
from concourse.bass_utils import run_bass_kernel_spmd

F32 = mybir.dt.float32
F32R = mybir.dt.float32r
BF16 = mybir.dt.bfloat16

B, S, H = 2, 2048, 2048
NH, HD = 16, 128
NC = 8
BS = B * S          # 4096 rows total
FL = H // NC        # 256 features per core (2 heads)
HL = NH // NC       # 2 heads per core
RPC = BS // NC      # 512 output rows per core
K16 = H // 128      # 16 contraction tiles
CW = 512            # phase-A row-chunk width
NCHUNK = BS // CW   # 8
QC = 512            # attention q-chunk width
SCALE = 1.0 / float(np.sqrt(HD))

_CACHED = None


def _build():
    nc = bacc.Bacc("TRN2", target_bir_lowering=False, debug=False, num_devices=NC)

    xT_d = nc.dram_tensor("xT", [H, BS], F32, kind="ExternalInput")
    wqT_d = nc.dram_tensor("wqT", [H, FL], F32, kind="ExternalInput")
    wkT_d = nc.dram_tensor("wkT", [H, FL], F32, kind="ExternalInput")
    wvT_d = nc.dram_tensor("wvT", [H, FL], F32, kind="ExternalInput")
    bq_d = nc.dram_tensor("bq", [128, HL], F32, kind="ExternalInput")
    bk_d = nc.dram_tensor("bk", [128, HL], F32, kind="ExternalInput")
    bv_d = nc.dram_tensor("bv_bc", [128, FL], F32, kind="ExternalInput")
    woT_d = nc.dram_tensor("woT", [H, H], F32, kind="ExternalInput")
    bo_d = nc.dram_tensor("bo_bc", [128, H], F32, kind="ExternalInput")
    onesb_d = nc.dram_tensor("ones_bf", [128, 128], BF16, kind="ExternalInput")
    out_d = nc.dram_tensor("out", [RPC, H], F32, kind="ExternalOutput")

    with tile.TileContext(nc) as tc:
        with (
            tc.tile_pool(name="consts", bufs=1) as cstp,
            tc.tile_pool(name="dram", bufs=1, space="DRAM") as dp,
            tc.tile_pool(name="stg", bufs=1) as stg,
            tc.tile_pool(name="woPre", bufs=1) as wcp,
            tc.tile_pool(name="psum", bufs=1, space="PSUM") as pp,
        ):
            ones_bf = cstp.tile([128, 128], BF16)
            nc.sync.dma_start(ones_bf[:], onesb_d.ap()[:])

            a2a_in = [dp.tile([NC, 128, RPC], BF16, name=f"a2a_in{h}") for h in range(HL)]
            a2a_out = [dp.tile([NC, 128, RPC], BF16, name=f"a2a_out{h}") for h in range(HL)]

            def cast_load(dst_slice, src_ap, width):
                """DMA f32 -> small staging, DVE-cast into bf16 dst slice."""
                src = stg.tile([128, CW], F32, tag="stg", bufs=6)
                nc.sync.dma_start(src[:, :width], src_ap)
                nc.vector.tensor_copy(dst_slice, src[:, :width])

            with tc.tile_pool(name="qkv", bufs=1) as qkvp:
                qT_sb = qkvp.tile([128, HL * BS], BF16)
                kT_sb = qkvp.tile([128, HL * BS], BF16)
                v_sb = qkvp.tile([128, (BS // 128) * FL], BF16)

                # ---------------- Phase A: QKV projections ----------------
                with (
                    tc.tile_pool(name="wgt", bufs=1) as wp,
                    tc.tile_pool(name="xbf", bufs=2) as xbp,
                ):
                    def load_w(dram):
                        dst = wp.tile([128, K16 * FL], BF16, tag=f"w_{dram.name}")
                        for k in range(K16):
                            cast_load(
                                dst[:, k * FL:(k + 1) * FL],
                                dram.ap()[k * 128:(k + 1) * 128, :],
                                FL,
                            )
                        return dst

                    def load_x(c):
                        dst = xbp.tile([128, K16 * CW], BF16, tag="xbf")
                        for k in range(K16):
                            cast_load(
                                dst[:, k * CW:(k + 1) * CW],
                                xT_d.ap()[k * 128:(k + 1) * 128,
                                          c * CW:(c + 1) * CW],
                                CW,
                            )
                        return dst

                    wq_sb = wp.tile([128, K16 * FL], BF16, tag="w_wqT")
                    xc0 = xbp.tile([128, K16 * CW], BF16, tag="xbf")
                    for k in range(K16):
                        cast_load(
                            wq_sb[:, k * FL:(k + 1) * FL],
                            wqT_d.ap()[k * 128:(k + 1) * 128, :],
                            FL,
                        )
                        cast_load(
                            xc0[:, k * CW:(k + 1) * CW],
                            xT_d.ap()[k * 128:(k + 1) * 128, :CW],
                            CW,
                        )
                    wk_sb = load_w(wkT_d)
                    wv_sb = load_w(wvT_d)

                    bq_sb = wp.tile([128, HL], F32)
                    nc.sync.dma_start(bq_sb[:], bq_d.ap()[:])
                    bk_sb = wp.tile([128, HL], F32)
                    nc.sync.dma_start(bk_sb[:], bk_d.ap()[:])
                    bv_sb = wp.tile([128, FL], F32)
                    nc.sync.dma_start(bv_sb[:], bv_d.ap()[:])

                    for c in range(NCHUNK):
                        xc = xc0 if c == 0 else load_x(c)
                        for w_sb, b_sb, dst in (
                            (wq_sb, bq_sb, qT_sb),
                            (wk_sb, bk_sb, kT_sb),
                        ):
                            for m in range(HL):
                                ps = pp.tile([128, CW], F32, tag="pss", bufs=3)
                                for k in range(K16):
                                    nc.tensor.matmul(
                                        ps[:],
                                        w_sb[:, k * FL + m * 128:
                                             k * FL + (m + 1) * 128],
                                        xc[:, k * CW:(k + 1) * CW],
                                        start=(k == 0),
                                        stop=(k == K16 - 1),
                                    )
                                nc.vector.tensor_scalar_add(
                                    dst[:, m * BS + c * CW: m * BS + (c + 1) * CW],
                                    ps[:],
                                    b_sb[:, m:m + 1],
                                )
                        for m2 in range(CW // 128):
                            ps = pp.tile([128, FL], F32, tag="psa", bufs=2)
                            for k in range(K16):
                                nc.tensor.matmul(
                                    ps[:],
                                    xc[:, k * CW + m2 * 128: k * CW + (m2 + 1) * 128],
                                    wv_sb[:, k * FL:(k + 1) * FL],
                                    start=(k == 0),
                                    stop=(k == K16 - 1),
                                )
                            i = c * (CW // 128) + m2
                            nc.vector.tensor_add(
                                v_sb[:, i * FL:(i + 1) * FL], ps[:], bv_sb[:]
                            )

                # prefetch Wo (cast to bf16) during attention
                won_tiles = {}
                for n in range(2):
                    won = wcp.tile([128, K16 * 512], BF16, tag="won", bufs=2)
                    for k in range(K16):
                        cast_load(
                            won[:, k * 512:(k + 1) * 512],
                            woT_d.ap()[k * 128:(k + 1) * 128,
                                       n * 512:(n + 1) * 512],
                            512,
                        )
                    won_tiles[n] = won

                # ---------------- Phase B: attention ----------------
                with (
                    tc.tile_pool(name="expp", bufs=3) as ep,
                    tc.tile_pool(name="tree", bufs=2) as trp,
                    tc.tile_pool(name="attp", bufs=2) as ap_,
                    tc.tile_pool(name="recp", bufs=2) as rp,
                ):
                    for h in range(HL):
                        for b in range(B):
                            base = h * BS + b * S
                            for qc in range(S // QC):
                                dest = b * (S // QC) + qc
                                expT = ep.tile([128, K16 * QC], BF16, tag="expT")
                                for km in range(K16):
                                    pss = pp.tile([128, QC], F32, tag="pss", bufs=3)
                                    nc.tensor.matmul(
                                        pss[:],
                                        kT_sb[:, base + km * 128:
                                              base + (km + 1) * 128],
                                        qT_sb[:, base + qc * QC:
                                              base + (qc + 1) * QC],
                                        start=True,
                                        stop=True,
                                    )
                                    nc.scalar.activation(
                                        expT[:, km * QC:(km + 1) * QC],
                                        pss[:],
                                        mybir.ActivationFunctionType.Exp,
                                        scale=SCALE,
                                    )
                                s1 = trp.tile([128, 8 * QC], BF16, tag="s1")
                                nc.vector.tensor_add(
                                    s1[:], expT[:, :8 * QC], expT[:, 8 * QC:]
                                )
                                s2 = trp.tile([128, 4 * QC], BF16, tag="s2")
                                nc.vector.tensor_add(
                                    s2[:], s1[:, :4 * QC], s1[:, 4 * QC:]
                                )
                                s3 = trp.tile([128, 2 * QC], BF16, tag="s3")
                                nc.vector.tensor_add(
                                    s3[:], s2[:, :2 * QC], s2[:, 2 * QC:]
                                )
                                s4 = trp.tile([128, QC], BF16, tag="s4")
                                nc.vector.tensor_add(s4[:], s3[:, :QC], s3[:, QC:])

                                psa = pp.tile([128, QC], F32, tag="psa", bufs=2)
                                for km in range(K16):
                                    nc.tensor.matmul(
                                        psa[:],
                                        v_sb[:, (16 * b + km) * FL + h * 128:
                                             (16 * b + km) * FL + (h + 1) * 128],
                                        expT[:, km * QC:(km + 1) * QC],
                                        start=(km == 0),
                                        stop=(km == K16 - 1),
                                    )
                                pssum = pp.tile([1, QC], F32, tag="pssum", bufs=1)
                                nc.tensor.matmul(
                                    pssum[:1, :], ones_bf[:, :1], s4[:],
                                    start=True, stop=True,
                                )
                                recip = rp.tile([1, QC], BF16, tag="recip")
                                with nc.allow_low_precision(
                                    reason="bf16 recip; rb rounds to bf16 anyway"
                                ):
                                    nc.vector.reciprocal(recip[:1, :], pssum[:1, :])
                                psb = pp.tile([128, QC], F32, tag="psb", bufs=1)
                                nc.tensor.matmul(
                                    psb[:],
                                    ones_bf[:1, :],
                                    recip[:1, :],
                                    start=True,
                                    stop=True,
                                )
                                rb = rp.tile([128, QC], BF16, tag="rb")
                                nc.vector.tensor_copy(rb[:], psb[:])
                                att = ap_.tile([128, QC], BF16, tag="att")
                                nc.vector.tensor_mul(att[:], psa[:], rb[:])
                                nc.gpsimd.dma_start(
                                    a2a_in[h][dest, :, :], att[:]
                                )

                        nc.gpsimd.collective_compute(
                            "AllToAll",
                            mybir.AluOpType.bypass,
                            ins=[a2a_in[h].opt()],
                            outs=[a2a_out[h].opt()],
                            replica_groups=[list(range(NC))],
                        )

            # ---------------- Phase C: output projection ----------------
            with (
                tc.tile_pool(name="aT", bufs=1) as atp,
                tc.tile_pool(name="boC", bufs=1) as bcp,
                tc.tile_pool(name="outC", bufs=3) as ocp,
            ):
                aT = atp.tile([128, K16 * RPC], BF16)
                for g in range(K16):
                    nc.sync.dma_start(
                        aT[:, g * RPC:(g + 1) * RPC],
                        a2a_out[g % 2][g // 2, :, :],
                    )
                bo_sb = bcp.tile([128, H], F32)
                nc.sync.dma_start(bo_sb[:], bo_d.ap()[:])
                for n in range(H // 512):
                    if n in won_tiles:
                        won = won_tiles[n]
                    else:
                        won = wcp.tile([128, K16 * 512], BF16, tag="won", bufs=2)
                        for k in range(K16):
                            cast_load(
                                won[:, k * 512:(k + 1) * 512],
                                woT_d.ap()[k * 128:(k + 1) * 128,
                                           n * 512:(n + 1) * 512],
                                512,
                            )
                    for m in range(RPC // 128):
                        pso = pp.tile([128, 512], F32, tag="pss", bufs=3)
                        for k in range(K16):
                            nc.tensor.matmul(
                                pso[:],
                                aT[:, k * RPC + m * 128: k * RPC + (m + 1) * 128],
                                won[:, k * 512:(k + 1) * 512],
                                start=(k == 0),
                                stop=(k == K16 - 1),
                            )
                        ot = ocp.tile([128, 512], F32, tag="ot")
                        nc.vector.tensor_add(
                            ot[:], pso[:], bo_sb[:, n * 512:(n + 1) * 512]
                        )
                        nc.sync.dma_start(
                            out_d.ap()[m * 128:(m + 1) * 128,
                                       n * 512:(n + 1) * 512],
                            ot[:],
                        )

    nc.compile()
    return nc


def _get_nc():
    global _CACHED
    if _CACHED is None:
        _CACHED = _build()
    return _CACHED


def _prep_in_maps(x, Wq, bq, Wk, bk, Wv, bv, Wo, bo):
    import ml_dtypes

    xT = np.ascontiguousarray(x.reshape(BS, H).T)
    woT = np.ascontiguousarray(Wo.T)
    bo_bc = np.ascontiguousarray(np.broadcast_to(bo, (128, H)))
    ones_bf = np.ones((128, 128), ml_dtypes.bfloat16)
    in_maps = []
    for c in range(NC):
        sl = slice(FL * c, FL * (c + 1))
        in_maps.append(
            {
                "xT": xT,
                "wqT": np.ascontiguousarray(Wq[sl, :].T),
                "wkT": np.ascontiguousarray(Wk[sl, :].T),
                "wvT": np.ascontiguousarray(Wv[sl, :].T),
                "bq": np.ascontiguousarray(bq[sl].reshape(HL, 128).T),
                "bk": np.ascontiguousarray(bk[sl].reshape(HL, 128).T),
                "bv_bc": np.ascontiguousarray(np.broadcast_to(bv[sl], (128, FL))),
                "woT": woT,
                "bo_bc": bo_bc,
                "ones_bf": ones_bf,
            }
        )
    return in_maps


def run(in_maps, trace=False):
    nc = _get_nc()
    return run_bass_kernel_spmd(nc, in_maps, core_ids=list(range(NC)), trace=trace)


def kernel(x, Wq, bq, Wk, bk, Wv, bv, Wo, bo):
    args = [np.asarray(a, dtype=np.float32) for a in (x, Wq, bq, Wk, bk, Wv, bv, Wo, bo)]
    in_maps = _prep_in_maps(*args)
    res = run(in_maps)
    out = np.concatenate([res.results[c]["out"] for c in range(NC)], axis=0)
    return out.reshape(B, S, H)
